# revision 36
# baseline (speedup 1.0000x reference)
"""Trainium2 Bass kernel for nn_LinearTriParser (B=2,S=128,H=1024,A=256,C=14).

Math: score[b,i,j,k,c] = sh0[i,c]+st0[j,c]+sm0[k,c]; softmax over k with
mask k in [i,j]. sh0/st0 are constant in k so alpha depends only on sm0:
  valid (i<=j): alpha = exp(sm0[k])/sum_{k'=i..j} exp(sm0[k'])
  invalid (i>j): all scores masked => alpha uniform = 1/S
final[b,i,j,c] = sh1[i,c]+st1[j,c]+uni[c] + sum_k alpha*sm1[k,c]
With P0=cumsum(exp(sm0)) and P1'' = cumsum(exp(sm0)*(sm1-mean(sm1))):
  den = P0[j]-P0[i-1];  attn = mean + (P1''[j]-P1''[i-1])/den  (valid)
  attn = mean(sm1)                                             (invalid)

Device structure per core (8 cores x identical SPMD program):
 - host-folded W2@sW kills layer 2 + W2 DMA; host-pretransposed memory
   kills all PE transposes of X; bf16 MLP; f32r cubic matmuls (1 cyc/row
   at N=448, fp32 precision for the P0[j]-P0[i-1] cancellation).
 - PE warm-up matmuls + act-table prefetch at t=0 (p-state / table load
   off the critical path).
 - blob1 split in halves so the m branch starts mid-DMA; interleaved
   PSUM accumulation groups per a-tile half.
 - denominator guard: max(den, 0.25) (invalid windows have den <= 0).

Sharding: (batch b, j-quarter) per core; per-core behavior arrives only
as data (mxt row slice, jselT, mask) + host reassembly.
"""

import numpy as np

B, S, H, A, C = 2, 128, 1024, 256, 14
P = 128
JW = 32            # j columns per core
W = JW * C         # 448 free width of cubic tiles
KH = H // P        # 8 k-tiles over the H contraction

NB1A = 1648        # blob1a: [fh 112 | mx k0-3 512 | w1m k0-3 1024]
NB1B = 1536        # blob1b: [mx k4-7 512 | w1m k4-7 1024]

# --- fp8 blob2 (mxt + w1t) offsets ---
MXT0 = 0           # [128, 8*32] memory^T row-slice (this core's j block)
W1T0 = MXT0 + 256  # [128, 8*256] t_W1 tiles
NB2 = W1T0 + 2048
# --- bf16 mask tensor extras ---
TRI0 = W            # [128, 32] trisel[k,q] = k <= j0+q (fused cumsum+jsel)
EYE0 = W + 32       # [14, 14] bf16 eye (transpose identity)
NMSK = W + 46

# --- f32 sf tensor [128, 56] ---
SB1 = 0            # cols 0:6 = b1 per (branch h,t,m) x (a-tile 0,1)
SEYE = 6           # cols 6:20 = eye(14)
SCB = 20           # cols 20:24 = c0m | c1m | c1h+uni | c1t  (rows 0:14)
SJS = 24           # cols 24:56 = jselT (jselT[p, q] = p == j0+q)
NSF = 56

CW = 3 * W + 3 * P  # f32r const/cubic tensor width


def _build():
    import concourse.mybir as mybir
    import concourse.tile as tile
    from concourse import bacc

    f32 = mybir.dt.float32
    f32r = mybir.dt.float32r
    bf16 = mybir.dt.bfloat16
    f8 = mybir.dt.float8e4
    AF = mybir.ActivationFunctionType
    OP = mybir.AluOpType

    nc = bacc.Bacc("TRN2", target_bir_lowering=False, debug=False,
                   enable_asserts=False, num_devices=8)

    blob1a = nc.dram_tensor("blob1a", [P, NB1A], f8, kind="ExternalInput")
    blob1b = nc.dram_tensor("blob1b", [P, NB1B], f8, kind="ExternalInput")
    blob2 = nc.dram_tensor("blob2", [P, NB2], f8, kind="ExternalInput")
    w1h = nc.dram_tensor("w1h", [P, 8 * A], f8, kind="ExternalInput")
    sf = nc.dram_tensor("sf", [P, NSF], f32, kind="ExternalInput")
    cc = nc.dram_tensor("cc", [15, CW], f32r, kind="ExternalInput")
    maskd = nc.dram_tensor("maskd", [P, NMSK], bf16, kind="ExternalInput")
    outp = nc.dram_tensor("outp", [P, W], bf16, kind="ExternalOutput")

    with tile.TileContext(nc) as tc:
        with (
            tc.tile_pool(name="pers", bufs=1) as pers,
            tc.tile_pool(name="work", bufs=2) as work,
            tc.tile_pool(name="ps_a", bufs=2, space="PSUM") as ps_a,
            tc.tile_pool(name="ps_b", bufs=2, space="PSUM") as ps_b,
            tc.tile_pool(name="ps_hd", bufs=2, space="PSUM") as ps_hd,
            tc.tile_pool(name="ps_big", bufs=2, space="PSUM") as ps_big,
        ):
            ps_l1 = (ps_a, ps_b)
            # ---- t=0 prefetches: act table load + PE p-state anchor ----
            dum = pers.tile([1, 4], f32, name="dum", tag="dum")
            nc.vector.memset(dum[:], 1.0)
            nc.scalar.activation(dum[:, 0:1], dum[:, 0:1], AF.Exp, scale=1.0)
            pdum = ps_hd.tile([1, 1], f32, name="pdum", tag="hdps")
            for _ in range(3):
                nc.tensor.matmul(pdum[:], dum[:, 1:2], dum[:, 2:3],
                                 start=True, stop=True)

            # ---- input DMAs (order = issue order on SP/HWDGE) ----
            b1_sb = pers.tile([P, NB1A + NB1B], f8, name="b1sb", tag="b1sb")
            nc.sync.dma_start(b1_sb[:, 0:NB1A], blob1a.ap())
            nc.sync.dma_start(b1_sb[:, NB1A:NB1A + NB1B], blob1b.ap())
            sf_sb = pers.tile([P, NSF], f32, name="sfsb", tag="sfsb")
            nc.sync.dma_start(sf_sb[:], sf.ap())
            b2_sb = pers.tile([P, NB2], f8, name="b2sb", tag="b2sb")
            nc.sync.dma_start(b2_sb[:], blob2.ap())
            mask_sb = pers.tile([P, NMSK], bf16, name="mask", tag="mask")
            nc.sync.dma_start(mask_sb[:], maskd.ap())
            wh_sb = pers.tile([P, 8 * A], f8, name="whsb", tag="whsb")
            nc.sync.dma_start(wh_sb[:], w1h.ap())
            cc_sb = pers.tile([15, CW], f32r, name="rhs", tag="rhs")
            nc.sync.dma_start(cc_sb[:], cc.ap())
            lhsT = cc_sb[:, 3 * W:3 * W + 3 * P]

            fh = b1_sb[:, 0:112]
            trisel = mask_sb[:, TRI0:TRI0 + JW]
            eye8 = mask_sb[0:C, EYE0:EYE0 + C]
            mxt = b2_sb[:, MXT0:MXT0 + 256]
            wt = b2_sb[:, W1T0:W1T0 + 2048]

            def mx_k(k):
                base = 112 if k < 4 else NB1A
                return b1_sb[:, base + (k % 4) * P: base + (k % 4) * P + P]

            def w1m_k(k, kk):
                base = 624 if k < 4 else NB1A + 512
                c0 = base + (k % 4) * A + kk * P
                return b1_sb[:, c0:c0 + P]

            # ---- m branch: interleaved kk groups, starts after blob1a ----
            ym = pers.tile([P, 2 * P], f8, name="ym", tag="ym")
            psm = [ps_l1[kk].tile([P, P], f32, name=f"psm{kk}", tag=f"l1ps{kk}")
                   for kk in range(2)]
            for k in range(KH):
                for kk in range(2):
                    nc.tensor.matmul(psm[kk][:], w1m_k(k, kk), mx_k(k),
                                     start=(k == 0), stop=(k == KH - 1))
            nc.scalar.activation(ym[:, 0:P], psm[0][:], AF.Relu,
                                 bias=sf_sb[:, 4:5], scale=1.0)
            nc.vector.tensor_scalar(ym[:, P:2 * P], psm[1][:],
                                    sf_sb[:, 5:6], 0.0, op0=OP.add, op1=OP.max)

            def head(yap, n, f0, bcol, nm, accum=None):
                ph = ps_hd.tile([C, n], f32, name=f"ph{nm}", tag="hdps")
                for kk in range(2):
                    nc.tensor.matmul(
                        ph[:], fh[:, f0 + kk * C: f0 + (kk + 1) * C],
                        yap[:, kk * n:(kk + 1) * n],
                        start=(kk == 0), stop=(kk == 1))
                ev = pers.tile([C, n], f32, name=f"se{nm}", tag=f"se{nm}")
                nc.scalar.activation(
                    ev[:], ph[:], AF.Identity,
                    bias=sf_sb[0:C, bcol:bcol + 1], scale=1.0 / 1024.0,
                    accum_out=accum)
                return ev

            # eE = exp(sm0 + c0m) fused straight from the head PSUM; eE
            # ordered before the sm1 head so Act never idles
            ph0 = ps_hd.tile([C, P], f32, name="phm0", tag="hdps")
            for kk in range(2):
                nc.tensor.matmul(
                    ph0[:], fh[:, 56 + kk * C: 56 + (kk + 1) * C],
                    ym[:, kk * P:(kk + 1) * P],
                    start=(kk == 0), stop=(kk == 1))
            ees = work.tile([C, 2 * P], bf16, name="ees", tag="ees")
            nc.scalar.activation(ees[:, 0:P], ph0[:], AF.Exp,
                                 bias=sf_sb[0:C, SCB:SCB + 1],
                                 scale=1.0 / 1024.0)
            ssum = work.tile([C, 1], f32, name="ssum", tag="ssum")
            sm1e = head(ym, P, 84, SCB + 1, "m1", accum=ssum[:])
            meanc = work.tile([C, 1], f32, name="mnc", tag="mnc")
            nc.scalar.activation(meanc[:], ssum[:], AF.Copy, scale=1.0 / P)

            # ---- t branch (this core's 32 j rows) -> st1 [14, 32] ----
            yt = pers.tile([P, 2 * JW], f8, name="yt", tag="yt")
            for kk in range(2):
                pst = ps_l1[kk].tile([P, JW], f32, name=f"pst{kk}",
                                     tag=f"l1ps{kk}")
                for k in range(KH):
                    nc.tensor.matmul(
                        pst[:], wt[:, k * A + kk * P: k * A + kk * P + P],
                        mxt[:, k * JW:(k + 1) * JW],
                        start=(k == 0), stop=(k == KH - 1))
                if kk == 0:
                    nc.scalar.activation(yt[:, 0:JW], pst[:], AF.Relu,
                                         bias=sf_sb[:, 2:3], scale=1.0)
                else:
                    nc.vector.tensor_scalar(yt[:, JW:2 * JW], pst[:],
                                            sf_sb[:, 3:4], 0.0,
                                            op0=OP.add, op1=OP.max)
            st1e = head(yt, JW, 28, SCB + 3, "t")

            # ---- prefix machinery ----
            # eS = (sm1 - mean) * eE in one op (fp8 out: the trisel matmul
            # and the DVE scans must sum identical rounded values)
            nc.vector.scalar_tensor_tensor(
                ees[:, P:2 * P], sm1e[:], meanc[:], ees[:, 0:P],
                op0=OP.subtract, op1=OP.mult)

            # ---- rhs row 14 via fused cumsum+select matmul ----
            pe8 = ps_big.tile([P, 2 * C], bf16, name="pe8", tag="big")
            nc.tensor.transpose(pe8[:, 0:C], ees[:, 0:P], eye8)
            nc.tensor.transpose(pe8[:, C:2 * C], ees[:, P:2 * P], eye8)
            ee2 = work.tile([P, 2 * C], bf16, name="ee2", tag="ee2")
            nc.vector.tensor_copy(ee2[:], pe8[:])
            pj = ps_big.tile([3 * JW, C], f32, name="pj", tag="big")
            nc.tensor.matmul(pj[0:JW, :], trisel, ee2[:, 0:C],
                             start=True, stop=True)
            nc.tensor.matmul(pj[JW:2 * JW, :], trisel, ee2[:, C:2 * C],
                             start=True, stop=True)
            nc.tensor.matmul(pj[2 * JW:3 * JW, :], st1e[:],
                             sf_sb[0:C, SEYE:SEYE + C], start=True, stop=True)
            j3 = work.tile([3 * JW, C], f32r, name="j3", tag="j3")
            nc.vector.tensor_copy(j3[:], pj[:])
            nc.sync.dma_start(
                cc_sb[14:15, 0:3 * W].rearrange("p (a b) -> p a b", a=3 * JW),
                j3[:],
            )

            # ---- scans (feed only ld/ln now; off the rhs critical path) ----
            p0 = work.tile([C, P], f32, name="p0", tag="p0")
            nc.vector.tensor_tensor_scan(
                p0[:], ees[:, 0:P], ees[:, 0:P], 0.0,
                op0=OP.add, op1=OP.bypass)
            p1n = work.tile([C, P], f32, name="p1n", tag="p1n")
            nc.vector.tensor_tensor_scan(
                p1n[:], ees[:, P:2 * P], ees[:, P:2 * P], 0.0,
                op0=OP.add, op1=OP.bypass)

            # ---- lhsT ld/ln (independent of the h branch) ----
            # ld = -Z0 (shifted -P0)
            nc.vector.tensor_scalar_mul(lhsT[0:C, P + 1:2 * P],
                                        p0[:, 0:P - 1], -1.0)
            # ln = -Z1'' (shifted -P1'')
            nc.vector.tensor_scalar_mul(lhsT[0:C, 2 * P + 1:3 * P],
                                        p1n[:, 0:P - 1], -1.0)

            # ---- h branch -> sh1 [14, 128] ----
            yh = pers.tile([P, 2 * P], f8, name="yh", tag="yh")
            for kk in range(2):
                psh = ps_l1[kk].tile([P, P], f32, name=f"psh{kk}",
                                     tag=f"l1ps{kk}")
                for k in range(KH):
                    nc.tensor.matmul(
                        psh[:], wh_sb[:, k * A + kk * P: k * A + kk * P + P],
                        mx_k(k),
                        start=(k == 0), stop=(k == KH - 1))
                if kk == 0:
                    nc.scalar.activation(yh[:, 0:P], psh[:], AF.Relu,
                                         bias=sf_sb[:, 0:1], scale=1.0)
                else:
                    nc.vector.tensor_scalar(yh[:, P:2 * P], psh[:],
                                            sf_sb[:, 1:2], 0.0,
                                            op0=OP.add, op1=OP.max)
            sh1e = head(yh, P, 0, SCB + 2, "h")

            # lb = sh1 + (c1h+uni) + meanc  (waits on the h branch)
            nc.vector.tensor_scalar_add(lhsT[0:C, 0:P], sh1e[:], meanc[:])

            # ---- cubic matmuls [128, 448] + tail ----
            pD = ps_big.tile([P, W], f32, name="pD", tag="big")
            nc.tensor.matmul(pD[:], lhsT[:, P:2 * P], cc_sb[:, 0:W],
                             start=True, stop=True)
            pN = ps_big.tile([P, W], f32, name="pN", tag="big")
            nc.tensor.matmul(pN[:], lhsT[:, 2 * P:3 * P], cc_sb[:, W:2 * W],
                             start=True, stop=True)
            ds = work.tile([P, W], bf16, name="ds", tag="ds")
            nc.vector.tensor_scalar_max(ds[:], pD[:], 0.25)
            pns = work.tile([P, W], bf16, name="pns", tag="pns")
            nc.scalar.activation(pns[:], pN[:], AF.Copy, scale=1.0)
            rc = work.tile([P, W], bf16, name="rc", tag="rc")
            with nc.allow_low_precision("bf16 plenty for 2e-2 rel tol"):
                nc.vector.reciprocal(rc[:], ds[:])
            nm = work.tile([P, W], bf16, name="nm", tag="nm")
            nc.vector.tensor_mul(nm[:], pns[:], mask_sb[:, 0:W])
            pB = ps_big.tile([P, W], f32, name="pB", tag="big")
            nc.tensor.matmul(pB[:], lhsT[:, 0:P], cc_sb[:, 2 * W:3 * W],
                             start=True, stop=True)
            pbs = work.tile([P, W], bf16, name="pbs", tag="pbs")
            nc.scalar.activation(pbs[:], pB[:], AF.Copy, scale=1.0)
            at = work.tile([P, W], bf16, name="at", tag="at")
            nc.vector.tensor_mul(at[:], nm[:], rc[:])
            fin = work.tile([P, W], bf16, name="fin", tag="fin")
            nc.vector.tensor_add(fin[:], at[:], pbs[:])
            nc.sync.dma_start(outp.ap(), fin[:])

    nc.finalize()
    return nc


_NC_CACHE = None


def _tile8(w):
    """[H, X] -> [128, 8*X]: col block k = rows 128k:128k+128."""
    return np.ascontiguousarray(
        w.reshape(KH, P, -1).transpose(1, 0, 2).reshape(P, -1))


def kernel(**inputs):
    import ml_dtypes
    from concourse.bass_utils import run_bass_kernel_spmd

    global _NC_CACHE
    if _NC_CACHE is None:
        _NC_CACHE = _build()
    nc = _NC_CACHE

    bf16 = ml_dtypes.bfloat16
    f32 = np.float32
    m = {k: np.asarray(v, f32) for k, v in inputs.items()}
    memory = m["memory"]

    # host-folded layer-2 + score heads
    F1h = m["h_W2"] @ m["s1h_W"] * 64.0
    c1h = m["h_b2"] @ m["s1h_W"] + m["s1h_b"] + m["uni"]
    F1t = m["t_W2"] @ m["s1t_W"] * 64.0
    c1t = m["t_b2"] @ m["s1t_W"] + m["s1t_b"]
    F0m = m["m_W2"] @ m["s0m_W"] * 64.0
    c0m = m["m_b2"] @ m["s0m_W"] + m["s0m_b"]
    F1m = m["m_W2"] @ m["s1m_W"] * 64.0
    c1m = m["m_b2"] @ m["s1m_W"] + m["s1m_b"]

    fhp = np.concatenate(
        [F1h.reshape(2, P, C).transpose(1, 0, 2).reshape(P, 2 * C),
         F1t.reshape(2, P, C).transpose(1, 0, 2).reshape(P, 2 * C),
         F0m.reshape(2, P, C).transpose(1, 0, 2).reshape(P, 2 * C),
         F1m.reshape(2, P, C).transpose(1, 0, 2).reshape(P, 2 * C)],
        axis=1)  # [128, 112]

    f8 = ml_dtypes.float8_e4m3
    w1m_p = _tile8(m["m_W1"] * 16.0)
    w1t_p = _tile8(m["t_W1"] * 16.0)
    w1h_p = np.asarray(_tile8(m["h_W1"] * 16.0), f8)

    sfc = np.zeros((P, NSF), f32)
    for bi, br in enumerate("htm"):
        sfc[:, 2 * bi] = m[f"{br}_b1"][0:P] * 16.0
        sfc[:, 2 * bi + 1] = m[f"{br}_b1"][P:2 * P] * 16.0
    sfc[0:C, SEYE:SEYE + C] = np.eye(C, dtype=f32)
    sfc[0:C, SCB + 0] = c0m
    sfc[0:C, SCB + 1] = c1m
    sfc[0:C, SCB + 2] = c1h
    sfc[0:C, SCB + 3] = c1t

    comb = (np.arange(C)[:, None, None, None] ==
            np.arange(C)[None, None, None, :]).astype(f32)
    ccp = np.zeros((15, CW), f32)
    ccp[0:C, 0:3 * W] = np.broadcast_to(comb, (C, 3, JW, C)).reshape(C, 3 * W)
    ccp[14, 3 * W:CW] = 1.0   # row-14 broadcast rows for all three groups

    mxp = {}
    for b in range(B):
        mxp[b] = _tile8(np.ascontiguousarray(memory[b].T))  # [128, 1024]

    in_maps = []
    ii = np.arange(P)[:, None]
    for cid in range(8):
        b, jq = cid // 4, cid % 4
        j0 = jq * JW
        jg = j0 + np.arange(JW)
        msk = (jg[None, :, None] >= ii[:, :, None]).astype(f32)
        msk = np.broadcast_to(msk, (P, JW, C)).reshape(P, W)

        b1a = np.concatenate([fhp, mxp[b][:, 0:512], w1m_p[:, 0:1024]],
                             axis=1)
        b1b = np.concatenate([mxp[b][:, 512:1024], w1m_p[:, 1024:2048]],
                             axis=1)
        mxt = memory[b, j0:j0 + JW].T.reshape(KH, P, JW)
        mxt = mxt.transpose(1, 0, 2).reshape(P, 8 * JW)
        blob2 = np.concatenate([mxt, w1t_p], axis=1)
        tri = (np.arange(P)[:, None] <= (j0 + np.arange(JW))[None, :])
        tri = tri.astype(f32)
        eye14 = np.zeros((P, C), f32)
        eye14[0:C, 0:C] = np.eye(C, dtype=f32)
        mskx = np.concatenate([msk, tri, eye14], axis=1)

        sfi = sfc.copy()
        sfi[j0 + np.arange(JW), SJS + np.arange(JW)] = 1.0

        in_maps.append({
            "blob1a": np.asarray(b1a, f8),
            "blob1b": np.asarray(b1b, f8),
            "blob2": np.asarray(blob2, f8),
            "w1h": w1h_p,
            "sf": sfi,
            "cc": ccp,
            "maskd": np.asarray(mskx, bf16),
        })

    res = run_bass_kernel_spmd(nc, in_maps, core_ids=list(range(8)))
    out = np.zeros((B, S, S, C), dtype=f32)
    for cid in range(8):
        b, jq = cid // 4, cid % 4
        j0 = jq * JW
        out[b, :, j0:j0 + JW, :] = np.asarray(
            res.results[cid]["outp"], f32).reshape(P, JW, C)
    return out


# revision 37
# speedup vs baseline: 1.0098x; 1.0098x over previous
"""Trainium2 Bass kernel for nn_LinearTriParser (B=2,S=128,H=1024,A=256,C=14).

Math: score[b,i,j,k,c] = sh0[i,c]+st0[j,c]+sm0[k,c]; softmax over k with
mask k in [i,j]. sh0/st0 are constant in k so alpha depends only on sm0:
  valid (i<=j): alpha = exp(sm0[k])/sum_{k'=i..j} exp(sm0[k'])
  invalid (i>j): all scores masked => alpha uniform = 1/S
final[b,i,j,c] = sh1[i,c]+st1[j,c]+uni[c] + sum_k alpha*sm1[k,c]
With P0=cumsum(exp(sm0)) and P1'' = cumsum(exp(sm0)*(sm1-mean(sm1))):
  den = P0[j]-P0[i-1];  attn = mean + (P1''[j]-P1''[i-1])/den  (valid)
  attn = mean(sm1)                                             (invalid)

Device structure per core (8 cores x identical SPMD program):
 - host-folded W2@sW kills layer 2 + W2 DMA; host-pretransposed memory
   kills all PE transposes of X; bf16 MLP; f32r cubic matmuls (1 cyc/row
   at N=448, fp32 precision for the P0[j]-P0[i-1] cancellation).
 - PE warm-up matmuls + act-table prefetch at t=0 (p-state / table load
   off the critical path).
 - blob1 split in halves so the m branch starts mid-DMA; interleaved
   PSUM accumulation groups per a-tile half.
 - denominator guard: max(den, 0.25) (invalid windows have den <= 0).

Sharding: (batch b, j-quarter) per core; per-core behavior arrives only
as data (mxt row slice, jselT, mask) + host reassembly.
"""

import numpy as np

B, S, H, A, C = 2, 128, 1024, 256, 14
P = 128
JW = 32            # j columns per core
W = JW * C         # 448 free width of cubic tiles
KH = H // P        # 8 k-tiles over the H contraction

NB1A = 1648        # blob1a: [fh 112 | mx k0-3 512 | w1m k0-3 1024]
NB1B = 1536        # blob1b: [mx k4-7 512 | w1m k4-7 1024]

# --- fp8 blob2 (mxt + w1t) offsets ---
MXT0 = 0           # [128, 8*32] memory^T row-slice (this core's j block)
W1T0 = MXT0 + 256  # [128, 8*256] t_W1 tiles
NB2 = W1T0 + 2048
# --- bf16 mask tensor extras ---
TRI0 = W            # [128, 32] trisel[k,q] = k <= j0+q (fused cumsum+jsel)
EYE0 = W + 32       # [14, 14] bf16 eye (transpose identity)
NMSK = W + 46

# --- f32 sf tensor [128, 56] ---
SB1 = 0            # cols 0:6 = b1 per (branch h,t,m) x (a-tile 0,1)
SEYE = 6           # cols 6:20 = eye(14)
SCB = 20           # cols 20:24 = c0m | c1m | c1h+uni | c1t  (rows 0:14)
SJS = 24           # cols 24:56 = jselT (jselT[p, q] = p == j0+q)
NSF = 56

CW = 3 * W + 3 * P  # f32r const/cubic tensor width


def _build():
    import concourse.mybir as mybir
    import concourse.tile as tile
    from concourse import bacc

    f32 = mybir.dt.float32
    f32r = mybir.dt.float32r
    bf16 = mybir.dt.bfloat16
    f8 = mybir.dt.float8e4
    AF = mybir.ActivationFunctionType
    OP = mybir.AluOpType

    nc = bacc.Bacc("TRN2", target_bir_lowering=False, debug=False,
                   enable_asserts=False, num_devices=8)

    blob1a = nc.dram_tensor("blob1a", [P, NB1A], f8, kind="ExternalInput")
    blob1b = nc.dram_tensor("blob1b", [P, NB1B], f8, kind="ExternalInput")
    blob2 = nc.dram_tensor("blob2", [P, NB2], f8, kind="ExternalInput")
    w1h = nc.dram_tensor("w1h", [P, 8 * A], f8, kind="ExternalInput")
    sf = nc.dram_tensor("sf", [P, NSF], f32, kind="ExternalInput")
    cc = nc.dram_tensor("cc", [15, CW], f32r, kind="ExternalInput")
    maskd = nc.dram_tensor("maskd", [P, NMSK], bf16, kind="ExternalInput")
    outp = nc.dram_tensor("outp", [P, W], bf16, kind="ExternalOutput")

    with tile.TileContext(nc) as tc:
        with (
            tc.tile_pool(name="pers", bufs=1) as pers,
            tc.tile_pool(name="work", bufs=2) as work,
            tc.tile_pool(name="ps_a", bufs=2, space="PSUM") as ps_a,
            tc.tile_pool(name="ps_b", bufs=2, space="PSUM") as ps_b,
            tc.tile_pool(name="ps_hd", bufs=2, space="PSUM") as ps_hd,
            tc.tile_pool(name="ps_big", bufs=2, space="PSUM") as ps_big,
        ):
            ps_l1 = (ps_a, ps_b)
            # ---- t=0 prefetches: act table load + PE p-state anchor ----
            dum = pers.tile([1, 4], f32, name="dum", tag="dum")
            nc.vector.memset(dum[:], 1.0)
            nc.scalar.activation(dum[:, 0:1], dum[:, 0:1], AF.Exp, scale=1.0)
            pdum = ps_hd.tile([1, 1], f32, name="pdum", tag="hdps")
            for _ in range(3):
                nc.tensor.matmul(pdum[:], dum[:, 1:2], dum[:, 2:3],
                                 start=True, stop=True)

            # ---- input DMAs (order = issue order on SP/HWDGE) ----
            b1_sb = pers.tile([P, NB1A + NB1B], f8, name="b1sb", tag="b1sb")
            nc.sync.dma_start(b1_sb[:, 0:NB1A], blob1a.ap())
            nc.sync.dma_start(b1_sb[:, NB1A:NB1A + NB1B], blob1b.ap())
            sf_sb = pers.tile([P, NSF], f32, name="sfsb", tag="sfsb")
            nc.sync.dma_start(sf_sb[:], sf.ap())
            b2_sb = pers.tile([P, NB2], f8, name="b2sb", tag="b2sb")
            nc.sync.dma_start(b2_sb[:], blob2.ap())
            mask_sb = pers.tile([P, NMSK], bf16, name="mask", tag="mask")
            nc.sync.dma_start(mask_sb[:], maskd.ap())
            wh_sb = pers.tile([P, 8 * A], f8, name="whsb", tag="whsb")
            nc.sync.dma_start(wh_sb[:], w1h.ap())
            cc_sb = pers.tile([15, CW], f32r, name="rhs", tag="rhs")
            nc.sync.dma_start(cc_sb[:], cc.ap())
            lhsT = cc_sb[:, 3 * W:3 * W + 3 * P]

            fh = b1_sb[:, 0:112]
            trisel = mask_sb[:, TRI0:TRI0 + JW]
            eye8 = mask_sb[0:C, EYE0:EYE0 + C]
            mxt = b2_sb[:, MXT0:MXT0 + 256]
            wt = b2_sb[:, W1T0:W1T0 + 2048]

            def mx_k(k):
                base = 112 if k < 4 else NB1A
                return b1_sb[:, base + (k % 4) * P: base + (k % 4) * P + P]

            def w1m_k(k, kk):
                base = 624 if k < 4 else NB1A + 512
                c0 = base + (k % 4) * A + kk * P
                return b1_sb[:, c0:c0 + P]

            # ---- m branch: interleaved kk groups, starts after blob1a ----
            ym = pers.tile([P, 2 * P], f8, name="ym", tag="ym")
            psm = [ps_l1[kk].tile([P, P], f32, name=f"psm{kk}", tag=f"l1ps{kk}")
                   for kk in range(2)]
            for k in range(KH):
                for kk in range(2):
                    nc.tensor.matmul(psm[kk][:], w1m_k(k, kk), mx_k(k),
                                     start=(k == 0), stop=(k == KH - 1))
            nc.scalar.activation(ym[:, 0:P], psm[0][:], AF.Relu,
                                 bias=sf_sb[:, 4:5], scale=1.0)
            nc.vector.tensor_scalar(ym[:, P:2 * P], psm[1][:],
                                    sf_sb[:, 5:6], 0.0, op0=OP.add, op1=OP.max)

            def head(yap, n, f0, bcol, nm, accum=None):
                ph = ps_hd.tile([C, n], f32, name=f"ph{nm}", tag="hdps")
                for kk in range(2):
                    nc.tensor.matmul(
                        ph[:], fh[:, f0 + kk * C: f0 + (kk + 1) * C],
                        yap[:, kk * n:(kk + 1) * n],
                        start=(kk == 0), stop=(kk == 1))
                ev = pers.tile([C, n], f32, name=f"se{nm}", tag=f"se{nm}")
                nc.scalar.activation(
                    ev[:], ph[:], AF.Identity,
                    bias=sf_sb[0:C, bcol:bcol + 1], scale=1.0 / 1024.0,
                    accum_out=accum)
                return ev

            # eE = exp(sm0 + c0m) fused straight from the head PSUM; eE
            # ordered before the sm1 head so Act never idles
            ph0 = ps_hd.tile([C, P], f32, name="phm0", tag="hdps")
            for kk in range(2):
                nc.tensor.matmul(
                    ph0[:], fh[:, 56 + kk * C: 56 + (kk + 1) * C],
                    ym[:, kk * P:(kk + 1) * P],
                    start=(kk == 0), stop=(kk == 1))
            ees = work.tile([C, 2 * P], bf16, name="ees", tag="ees")
            nc.scalar.activation(ees[:, 0:P], ph0[:], AF.Exp,
                                 bias=sf_sb[0:C, SCB:SCB + 1],
                                 scale=1.0 / 1024.0)
            ssum = work.tile([C, 1], f32, name="ssum", tag="ssum")
            sm1e = head(ym, P, 84, SCB + 1, "m1", accum=ssum[:])
            meanc = work.tile([C, 1], f32, name="mnc", tag="mnc")
            nc.vector.tensor_scalar_mul(meanc[:], ssum[:], 1.0 / P)

            # ---- t branch (this core's 32 j rows) -> st1 [14, 32] ----
            yt = pers.tile([P, 2 * JW], f8, name="yt", tag="yt")
            for kk in range(2):
                pst = ps_l1[kk].tile([P, JW], f32, name=f"pst{kk}",
                                     tag=f"l1ps{kk}")
                for k in range(KH):
                    nc.tensor.matmul(
                        pst[:], wt[:, k * A + kk * P: k * A + kk * P + P],
                        mxt[:, k * JW:(k + 1) * JW],
                        start=(k == 0), stop=(k == KH - 1))
                if kk == 0:
                    nc.scalar.activation(yt[:, 0:JW], pst[:], AF.Relu,
                                         bias=sf_sb[:, 2:3], scale=1.0)
                else:
                    nc.vector.tensor_scalar(yt[:, JW:2 * JW], pst[:],
                                            sf_sb[:, 3:4], 0.0,
                                            op0=OP.add, op1=OP.max)
            st1e = head(yt, JW, 28, SCB + 3, "t")

            # ---- prefix machinery ----
            # eS = (sm1 - mean) * eE in one op (fp8 out: the trisel matmul
            # and the DVE scans must sum identical rounded values)
            nc.vector.scalar_tensor_tensor(
                ees[:, P:2 * P], sm1e[:], meanc[:], ees[:, 0:P],
                op0=OP.subtract, op1=OP.mult)

            # ---- rhs row 14 via fused cumsum+select matmul ----
            pe8 = ps_big.tile([P, 2 * C], bf16, name="pe8", tag="big")
            nc.tensor.transpose(pe8[:, 0:C], ees[:, 0:P], eye8)
            nc.tensor.transpose(pe8[:, C:2 * C], ees[:, P:2 * P], eye8)
            ee2 = work.tile([P, 2 * C], bf16, name="ee2", tag="ee2")
            nc.vector.tensor_copy(ee2[:], pe8[:])
            pj = ps_big.tile([3 * JW, C], f32, name="pj", tag="big")
            nc.tensor.matmul(pj[0:JW, :], trisel, ee2[:, 0:C],
                             start=True, stop=True)
            nc.tensor.matmul(pj[JW:2 * JW, :], trisel, ee2[:, C:2 * C],
                             start=True, stop=True)
            nc.tensor.matmul(pj[2 * JW:3 * JW, :], st1e[:],
                             sf_sb[0:C, SEYE:SEYE + C], start=True, stop=True)
            j3 = work.tile([3 * JW, C], f32r, name="j3", tag="j3")
            nc.vector.tensor_copy(j3[:], pj[:])
            nc.sync.dma_start(
                cc_sb[14:15, 0:3 * W].rearrange("p (a b) -> p a b", a=3 * JW),
                j3[:],
            )

            # ---- scans (feed only ld/ln now; off the rhs critical path) ----
            p0 = work.tile([C, P], f32, name="p0", tag="p0")
            nc.vector.tensor_tensor_scan(
                p0[:], ees[:, 0:P], ees[:, 0:P], 0.0,
                op0=OP.add, op1=OP.bypass)
            p1n = work.tile([C, P], f32, name="p1n", tag="p1n")
            nc.vector.tensor_tensor_scan(
                p1n[:], ees[:, P:2 * P], ees[:, P:2 * P], 0.0,
                op0=OP.add, op1=OP.bypass)

            # ---- lhsT ld/ln (independent of the h branch) ----
            # ld = -Z0 (shifted -P0)
            nc.vector.tensor_scalar_mul(lhsT[0:C, P + 1:2 * P],
                                        p0[:, 0:P - 1], -1.0)
            # ln = -Z1'' (shifted -P1'')
            nc.vector.tensor_scalar_mul(lhsT[0:C, 2 * P + 1:3 * P],
                                        p1n[:, 0:P - 1], -1.0)

            # ---- h branch -> sh1 [14, 128] ----
            yh = pers.tile([P, 2 * P], f8, name="yh", tag="yh")
            for kk in range(2):
                psh = ps_l1[kk].tile([P, P], f32, name=f"psh{kk}",
                                     tag=f"l1ps{kk}")
                for k in range(KH):
                    nc.tensor.matmul(
                        psh[:], wh_sb[:, k * A + kk * P: k * A + kk * P + P],
                        mx_k(k),
                        start=(k == 0), stop=(k == KH - 1))
                if kk == 0:
                    nc.scalar.activation(yh[:, 0:P], psh[:], AF.Relu,
                                         bias=sf_sb[:, 0:1], scale=1.0)
                else:
                    nc.vector.tensor_scalar(yh[:, P:2 * P], psh[:],
                                            sf_sb[:, 1:2], 0.0,
                                            op0=OP.add, op1=OP.max)
            sh1e = head(yh, P, 0, SCB + 2, "h")

            # lb = sh1 + (c1h+uni) + meanc  (waits on the h branch)
            nc.vector.tensor_scalar_add(lhsT[0:C, 0:P], sh1e[:], meanc[:])

            # ---- cubic matmuls [128, 448] + tail ----
            pD = ps_big.tile([P, W], f32, name="pD", tag="big")
            nc.tensor.matmul(pD[:], lhsT[:, P:2 * P], cc_sb[:, 0:W],
                             start=True, stop=True)
            pN = ps_big.tile([P, W], f32, name="pN", tag="big")
            nc.tensor.matmul(pN[:], lhsT[:, 2 * P:3 * P], cc_sb[:, W:2 * W],
                             start=True, stop=True)
            ds = work.tile([P, W], bf16, name="ds", tag="ds")
            nc.vector.tensor_scalar_max(ds[:], pD[:], 0.25)
            pns = work.tile([P, W], bf16, name="pns", tag="pns")
            nc.scalar.activation(pns[:], pN[:], AF.Copy, scale=1.0)
            rc = work.tile([P, W], bf16, name="rc", tag="rc")
            with nc.allow_low_precision("bf16 plenty for 2e-2 rel tol"):
                nc.vector.reciprocal(rc[:], ds[:])
            nm = work.tile([P, W], bf16, name="nm", tag="nm")
            nc.vector.tensor_mul(nm[:], pns[:], mask_sb[:, 0:W])
            pB = ps_big.tile([P, W], f32, name="pB", tag="big")
            nc.tensor.matmul(pB[:], lhsT[:, 0:P], cc_sb[:, 2 * W:3 * W],
                             start=True, stop=True)
            pbs = work.tile([P, W], bf16, name="pbs", tag="pbs")
            nc.scalar.activation(pbs[:], pB[:], AF.Copy, scale=1.0)
            at = work.tile([P, W], bf16, name="at", tag="at")
            nc.vector.tensor_mul(at[:], nm[:], rc[:])
            fin = work.tile([P, W], bf16, name="fin", tag="fin")
            nc.vector.tensor_add(fin[:], at[:], pbs[:])
            nc.sync.dma_start(outp.ap(), fin[:])

    nc.finalize()
    return nc


_NC_CACHE = None


def _tile8(w):
    """[H, X] -> [128, 8*X]: col block k = rows 128k:128k+128."""
    return np.ascontiguousarray(
        w.reshape(KH, P, -1).transpose(1, 0, 2).reshape(P, -1))


def kernel(**inputs):
    import ml_dtypes
    from concourse.bass_utils import run_bass_kernel_spmd

    global _NC_CACHE
    if _NC_CACHE is None:
        _NC_CACHE = _build()
    nc = _NC_CACHE

    bf16 = ml_dtypes.bfloat16
    f32 = np.float32
    m = {k: np.asarray(v, f32) for k, v in inputs.items()}
    memory = m["memory"]

    # host-folded layer-2 + score heads
    F1h = m["h_W2"] @ m["s1h_W"] * 64.0
    c1h = m["h_b2"] @ m["s1h_W"] + m["s1h_b"] + m["uni"]
    F1t = m["t_W2"] @ m["s1t_W"] * 64.0
    c1t = m["t_b2"] @ m["s1t_W"] + m["s1t_b"]
    F0m = m["m_W2"] @ m["s0m_W"] * 64.0
    c0m = m["m_b2"] @ m["s0m_W"] + m["s0m_b"]
    F1m = m["m_W2"] @ m["s1m_W"] * 64.0
    c1m = m["m_b2"] @ m["s1m_W"] + m["s1m_b"]

    fhp = np.concatenate(
        [F1h.reshape(2, P, C).transpose(1, 0, 2).reshape(P, 2 * C),
         F1t.reshape(2, P, C).transpose(1, 0, 2).reshape(P, 2 * C),
         F0m.reshape(2, P, C).transpose(1, 0, 2).reshape(P, 2 * C),
         F1m.reshape(2, P, C).transpose(1, 0, 2).reshape(P, 2 * C)],
        axis=1)  # [128, 112]

    f8 = ml_dtypes.float8_e4m3
    w1m_p = _tile8(m["m_W1"] * 16.0)
    w1t_p = _tile8(m["t_W1"] * 16.0)
    w1h_p = np.asarray(_tile8(m["h_W1"] * 16.0), f8)

    sfc = np.zeros((P, NSF), f32)
    for bi, br in enumerate("htm"):
        sfc[:, 2 * bi] = m[f"{br}_b1"][0:P] * 16.0
        sfc[:, 2 * bi + 1] = m[f"{br}_b1"][P:2 * P] * 16.0
    sfc[0:C, SEYE:SEYE + C] = np.eye(C, dtype=f32)
    sfc[0:C, SCB + 0] = c0m
    sfc[0:C, SCB + 1] = c1m
    sfc[0:C, SCB + 2] = c1h
    sfc[0:C, SCB + 3] = c1t

    comb = (np.arange(C)[:, None, None, None] ==
            np.arange(C)[None, None, None, :]).astype(f32)
    ccp = np.zeros((15, CW), f32)
    ccp[0:C, 0:3 * W] = np.broadcast_to(comb, (C, 3, JW, C)).reshape(C, 3 * W)
    ccp[14, 3 * W:CW] = 1.0   # row-14 broadcast rows for all three groups

    mxp = {}
    for b in range(B):
        mxp[b] = _tile8(np.ascontiguousarray(memory[b].T))  # [128, 1024]

    in_maps = []
    ii = np.arange(P)[:, None]
    for cid in range(8):
        b, jq = cid // 4, cid % 4
        j0 = jq * JW
        jg = j0 + np.arange(JW)
        msk = (jg[None, :, None] >= ii[:, :, None]).astype(f32)
        msk = np.broadcast_to(msk, (P, JW, C)).reshape(P, W)

        b1a = np.concatenate([fhp, mxp[b][:, 0:512], w1m_p[:, 0:1024]],
                             axis=1)
        b1b = np.concatenate([mxp[b][:, 512:1024], w1m_p[:, 1024:2048]],
                             axis=1)
        mxt = memory[b, j0:j0 + JW].T.reshape(KH, P, JW)
        mxt = mxt.transpose(1, 0, 2).reshape(P, 8 * JW)
        blob2 = np.concatenate([mxt, w1t_p], axis=1)
        tri = (np.arange(P)[:, None] <= (j0 + np.arange(JW))[None, :])
        tri = tri.astype(f32)
        eye14 = np.zeros((P, C), f32)
        eye14[0:C, 0:C] = np.eye(C, dtype=f32)
        mskx = np.concatenate([msk, tri, eye14], axis=1)

        sfi = sfc.copy()
        sfi[j0 + np.arange(JW), SJS + np.arange(JW)] = 1.0

        in_maps.append({
            "blob1a": np.asarray(b1a, f8),
            "blob1b": np.asarray(b1b, f8),
            "blob2": np.asarray(blob2, f8),
            "w1h": w1h_p,
            "sf": sfi,
            "cc": ccp,
            "maskd": np.asarray(mskx, bf16),
        })

    res = run_bass_kernel_spmd(nc, in_maps, core_ids=list(range(8)))
    out = np.zeros((B, S, S, C), dtype=f32)
    for cid in range(8):
        b, jq = cid // 4, cid % 4
        j0 = jq * JW
        out[b, :, j0:j0 + JW, :] = np.asarray(
            res.results[cid]["outp"], f32).reshape(P, JW, C)
    return out
